# revision 7
# baseline (speedup 1.0000x reference)
# Trainium2 Bass kernel for nn_DetectorKe: Gaussian-mixture detector
#
#   out_n = sum_m coefs_m * exp(-(x_n-c_m)^T A_m (x_n-c_m)),  A_m = L_m L_m^T
#
# Math used here:
#   (x-c)^T A (x-c) = xAx - 2 xAc + cAc,  xAx = ||L^T x||^2 = sum_k u_k^2
#   u[n, m, k] = sum_d L[m, d, k] x[n, d]      (one big matmul, contraction d)
#   psum_d[m, n] = -sum_k u^2 + 2 xAc - cAc + ln(coefs)   (PE selector-reduce)
#   out = sum_m exp(psum_d)                                (ACT exp + PE ones-reduce)
#
# Sharding: data-parallel over 8 NeuronCores, 16384 points each; weights
# replicated. Per core, points are streamed d-major (xT) through the PE as the
# moving operand against 8 stationary L-blocks [32, 128]; the squared PSUM
# blocks are reduced over k back on the PE with constant selector matrices, so
# the only elementwise pass is the PSUM->SBUF square evacuation (ACT/DVE split).

import os
from contextlib import ExitStack

import numpy as np

import bass_rust
import concourse.bass as bass
import concourse.tile as tile
from concourse import mybir
from concourse.bass_utils import run_bass_kernel_spmd


def _patched_drain_and_barrier(self, tick_clock, wait_clock):
    # This container's walrus rejects instructions carrying >2 sync waits
    # ("Too many sync wait commands" on the Tile tail Drain). Spill the
    # drain's waits onto single-wait NOPs on the in-order sync queue, then
    # emit the drain bare.
    nc = self.nc
    probe = mybir.InstNoOp(name=nc.get_next_instruction_name(), ins=[], outs=[],
                           engine=mybir.EngineType.SP)
    wait_clock.add_sem_waits(
        probe, bass_rust.ScopedClock({None: tick_clock.global_clock}))
    si = probe.sync_info
    waits = list(si.on_wait) if si is not None else []
    name2sem = {s.name: s for s in self.sems.allocated().values()}
    for w in waits:
        nm = getattr(w, "ant_name", None) or getattr(w, "name", None)
        n = nc.sync.nop()
        bass_rust.wait_op(n.ins, name2sem[nm], w.wait_value, "sem-ge", True)
    nc.sync.drain()
    nc.all_engine_barrier()
    popped = nc._tile_sem_poison_stack.pop()
    assert popped is self._sem_poison
    nc.clear_and_free_semaphores(list(self.sems.allocated().values()))
    nc.all_engine_barrier()


tile.TileContext._drain_and_barrier = _patched_drain_and_barrier


def _split_excess_waits(bir: dict) -> dict:
    # This walrus accepts at most 1 sync wait on a normal instruction and 2 on
    # an EventSemaphore. Tile attaches several waits directly; hoist the
    # excess onto EventSemaphore instructions prepended on the same (in-order)
    # engine queue.
    uid = [0]

    def fix_list(insts):
        out = []
        for ins in insts:
            si = ins.get("sync_info") if isinstance(ins, dict) else None
            waits = si.get("on_wait", []) if isinstance(si, dict) else []
            cap = 2 if ins.get("opcode") == "EventSemaphore" else 1
            if len(waits) > cap:
                spill, keep = waits[:-cap], waits[-cap:]
                for i in range(0, len(spill), 2):
                    uid[0] += 1
                    out.append({
                        "debug": ins.get("debug", 0),
                        "engine": ins["engine"],
                        "ins": [], "outs": [],
                        "name": f"WSP-{uid[0]}-{ins['name']}",
                        "opcode": "EventSemaphore",
                        "sync_info": {"on_update": [],
                                      "on_wait": spill[i:i + 2]},
                    })
                si["on_wait"] = keep
            out.append(ins)
        return out

    def walk(o):
        if isinstance(o, dict):
            for k, v in o.items():
                if k == "instructions" and isinstance(v, list):
                    o[k] = fix_list(v)
                else:
                    walk(v)
        elif isinstance(o, list):
            for v in o:
                walk(v)

    walk(bir)
    return bir


_orig_to_json_bytes = bass.Bass.to_json_bytes


def _patched_to_json_bytes(self):
    import json as _json
    raw = _orig_to_json_bytes(self)
    bir = _json.loads(raw)
    bir = _split_excess_waits(bir)
    return _json.dumps(bir).encode()


bass.Bass.to_json_bytes = _patched_to_json_bytes

N, M, D = 131072, 32, 32
NCORES = 8
NC_N = N // NCORES          # points per core
CHUNK = 512                 # points per n-chunk (one fp32 matmul free-dim)
NCHUNK = NC_N // CHUNK      # 32
NBLK = (M * D) // 128       # 8 blocks of 128 (m,k) pairs
XTILE = 4                   # xta split into 4 SBUF tiles for load/compute overlap
LAG = 3                     # software pipeline lag (w-matmul ahead of square)

F32 = mybir.dt.float32


def _build_program():
    nc = bass.Bass("TRN2", debug=False)

    xta_d = nc.dram_tensor("xta", [D + 1, NC_N], F32, kind="ExternalInput").ap()
    lw_d = nc.dram_tensor("lw", [D, NBLK, 128], F32, kind="ExternalInput").ap()
    acw2_d = nc.dram_tensor("acw2", [D + 1, M], F32, kind="ExternalInput").ap()
    sel_d = nc.dram_tensor("sel", [128, NBLK, M], F32, kind="ExternalInput").ap()
    ones_d = nc.dram_tensor("onesw", [M, 1], F32, kind="ExternalInput").ap()
    out_d = nc.dram_tensor("out", [1, NC_N], F32, kind="ExternalOutput").ap()

    with tile.TileContext(nc) as tc, ExitStack() as ctx:
        singles = ctx.enter_context(tc.tile_pool(name="singles", bufs=1))
        xpool = ctx.enter_context(tc.tile_pool(name="xpool", bufs=1))
        u2pool = ctx.enter_context(tc.tile_pool(name="u2", bufs=LAG + 2))
        epool = ctx.enter_context(tc.tile_pool(name="exp", bufs=2))
        wps = ctx.enter_context(tc.tile_pool(name="wps", bufs=LAG + 1, space="PSUM"))
        dps = ctx.enter_context(tc.tile_pool(name="dps", bufs=2, space="PSUM"))
        fps = ctx.enter_context(tc.tile_pool(name="fps", bufs=2, space="PSUM"))

        lw_sb = singles.tile([D, NBLK, 128], F32)
        nc.sync.dma_start(out=lw_sb, in_=lw_d)
        acw2_sb = singles.tile([D + 1, M], F32)
        nc.sync.dma_start(out=acw2_sb, in_=acw2_d)
        sel_sb = singles.tile([128, NBLK, M], F32)
        nc.sync.dma_start(out=sel_sb, in_=sel_d)
        ones_sb = singles.tile([M, 1], F32)
        nc.sync.dma_start(out=ones_sb, in_=ones_d)
        osb = singles.tile([1, NC_N], F32)

        xtiles = []
        xw = NC_N // XTILE
        for i in range(XTILE):
            xt = xpool.tile([D + 1, xw], F32, name=f"xt{i}")
            nc.sync.dma_start(out=xt, in_=xta_d[:, i * xw:(i + 1) * xw])
            xtiles.append(xt)

        def xslice(c, rows):
            ti, off = divmod(c * CHUNK, xw)
            return xtiles[ti][0:rows, off:off + CHUNK]

        TOT = NCHUNK * NBLK
        wtiles = {}
        d_ps = None
        sq_ctr = 0
        for t in range(TOT + LAG):
            if t < TOT:
                c, b = divmod(t, NBLK)
                w = wps.tile([128, CHUNK], F32, name="w")
                nc.tensor.matmul(w, lw_sb[:, b, :], xslice(c, D),
                                 start=True, stop=True)
                wtiles[t] = w
            if t >= LAG:
                t2 = t - LAG
                c2, b2 = divmod(t2, NBLK)
                w = wtiles.pop(t2)
                u2 = u2pool.tile([128, CHUNK], F32, name="u2")
                # PSUM evacuation + square (ACT; DVE can't dual-read PSUM)
                nc.scalar.square(u2, w)
                sq_ctr += 1
                if b2 == 0:
                    d_ps = dps.tile([M, CHUNK], F32, name="d")
                nc.tensor.matmul(d_ps, sel_sb[:, b2, :], u2,
                                 start=(b2 == 0), stop=False,
                                 skip_group_check=True)
                if b2 == NBLK - 1:
                    nc.tensor.matmul(d_ps, acw2_sb, xslice(c2, D + 1),
                                     start=False, stop=True,
                                     skip_group_check=True)
                    e_sb = epool.tile([M, CHUNK], F32, name="e")
                    nc.scalar.activation(e_sb, d_ps,
                                         mybir.ActivationFunctionType.Exp)
                    f_ps = fps.tile([1, CHUNK], F32, name="f")
                    nc.tensor.matmul(f_ps, ones_sb, e_sb, start=True, stop=True)
                    nc.scalar.copy(
                        osb[0:1, c2 * CHUNK:(c2 + 1) * CHUNK], f_ps)

        nc.sync.dma_start(out=out_d, in_=osb)
    return nc


_CACHE = {}


def _get_program():
    if "nc" not in _CACHE:
        _CACHE["nc"] = _build_program()
    return _CACHE["nc"]


def _host_prep(points, centers, covs_inv_sqrt, weights):
    L = covs_inv_sqrt.astype(np.float64)                    # [M, D, D]
    A = np.einsum("mdk,mek->mde", L, L)                     # [M, D, D]
    Ac = np.einsum("mde,me->md", A, centers.astype(np.float64))
    cAc = np.einsum("md,md->m", Ac, centers.astype(np.float64))
    wf = weights.astype(np.float64)
    sm = np.exp(wf - wf.max())
    sm /= sm.sum()
    coefs = sm * np.sqrt(np.linalg.det(A))
    lnc = np.log(coefs)

    lw = np.ascontiguousarray(
        L.transpose(1, 0, 2).reshape(D, NBLK, 128)).astype(np.float32)
    acw2 = np.zeros((D + 1, M), np.float32)
    acw2[:D, :] = (2.0 * Ac.T).astype(np.float32)
    acw2[D, :] = (-cAc + lnc).astype(np.float32)
    sel = np.zeros((128, NBLK, M), np.float32)
    p = np.arange(128)
    for b in range(NBLK):
        sel[p, b, 4 * b + p // 32] = -1.0
    onesw = np.ones((M, 1), np.float32)

    xta = np.empty((D + 1, N), np.float32)
    xta[:D, :] = points.T
    xta[D, :] = 1.0
    return xta, lw, acw2, sel, onesw


def kernel(points, centers, covs_inv_sqrt, weights):
    xta, lw, acw2, sel, onesw = _host_prep(points, centers, covs_inv_sqrt, weights)
    nc = _get_program()
    in_maps = []
    for i in range(NCORES):
        in_maps.append({
            "xta": np.ascontiguousarray(xta[:, i * NC_N:(i + 1) * NC_N]),
            "lw": lw, "acw2": acw2, "sel": sel, "onesw": onesw,
        })
    trace = os.environ.get("KBENCH_TRACE", "") == "1"
    if trace:
        try:
            from antenv.axon_hooks import get_axon_ntff_profile_hook  # noqa: F401
        except ImportError:
            trace = False
    res = run_bass_kernel_spmd(nc, in_maps, core_ids=list(range(NCORES)),
                               trace=trace)
    _CACHE["last_result"] = res
    out = np.concatenate([res.results[i]["out"].reshape(-1)
                          for i in range(NCORES)])
    return out.astype(np.float32)


# revision 9
# speedup vs baseline: 2.6181x; 2.6181x over previous
# Trainium2 Bass kernel for nn_DetectorKe: Gaussian-mixture detector
#
#   out_n = sum_m coefs_m * exp(-(x_n-c_m)^T A_m (x_n-c_m)),  A_m = L_m L_m^T
#
# Math used here:
#   (x-c)^T A (x-c) = xAx - 2 xAc + cAc,  xAx = ||L^T x||^2 = sum_k u_k^2
#   u[n, m, k] = sum_d L[m, d, k] x[n, d]      (one big matmul, contraction d)
#   psum_d[m, n] = -sum_k u^2 + 2 xAc - cAc + ln(coefs)   (PE selector-reduce)
#   out = sum_m exp(psum_d)                                (ACT exp + PE ones-reduce)
#
# Sharding: data-parallel over 8 NeuronCores, 16384 points each; weights
# replicated. Per core, points are streamed d-major (xT) through the PE as the
# moving operand against 8 stationary L-blocks [32, 128]; the squared PSUM
# blocks are reduced over k back on the PE with constant selector matrices, so
# the only elementwise pass is the PSUM->SBUF square evacuation (ACT/DVE split).

import os
from contextlib import ExitStack

import numpy as np

import bass_rust
import concourse.bass as bass
import concourse.tile as tile
from concourse import mybir
from concourse.bass_utils import run_bass_kernel_spmd


def _patched_drain_and_barrier(self, tick_clock, wait_clock):
    # This container's walrus rejects instructions carrying >2 sync waits
    # ("Too many sync wait commands" on the Tile tail Drain). Spill the
    # drain's waits onto single-wait NOPs on the in-order sync queue, then
    # emit the drain bare.
    nc = self.nc
    probe = mybir.InstNoOp(name=nc.get_next_instruction_name(), ins=[], outs=[],
                           engine=mybir.EngineType.SP)
    wait_clock.add_sem_waits(
        probe, bass_rust.ScopedClock({None: tick_clock.global_clock}))
    si = probe.sync_info
    waits = list(si.on_wait) if si is not None else []
    name2sem = {s.name: s for s in self.sems.allocated().values()}
    for w in waits:
        nm = getattr(w, "ant_name", None) or getattr(w, "name", None)
        n = nc.sync.nop()
        bass_rust.wait_op(n.ins, name2sem[nm], w.wait_value, "sem-ge", True)
    nc.sync.drain()
    nc.all_engine_barrier()
    popped = nc._tile_sem_poison_stack.pop()
    assert popped is self._sem_poison
    nc.clear_and_free_semaphores(list(self.sems.allocated().values()))
    nc.all_engine_barrier()


tile.TileContext._drain_and_barrier = _patched_drain_and_barrier


def _split_excess_waits(bir: dict) -> dict:
    # This walrus accepts at most 1 sync wait on a normal instruction and 2 on
    # an EventSemaphore. Tile attaches several waits directly; hoist the
    # excess onto EventSemaphore instructions prepended on the same (in-order)
    # engine queue.
    uid = [0]

    def fix_list(insts):
        out = []
        for ins in insts:
            si = ins.get("sync_info") if isinstance(ins, dict) else None
            waits = si.get("on_wait", []) if isinstance(si, dict) else []
            cap = 2 if ins.get("opcode") == "EventSemaphore" else 1
            if len(waits) > cap:
                spill, keep = waits[:-cap], waits[-cap:]
                for i in range(0, len(spill), 2):
                    uid[0] += 1
                    out.append({
                        "debug": ins.get("debug", 0),
                        "engine": ins["engine"],
                        "ins": [], "outs": [],
                        "name": f"WSP-{uid[0]}-{ins['name']}",
                        "opcode": "EventSemaphore",
                        "sync_info": {"on_update": [],
                                      "on_wait": spill[i:i + 2]},
                    })
                si["on_wait"] = keep
            out.append(ins)
        return out

    def walk(o):
        if isinstance(o, dict):
            for k, v in o.items():
                if k == "instructions" and isinstance(v, list):
                    o[k] = fix_list(v)
                else:
                    walk(v)
        elif isinstance(o, list):
            for v in o:
                walk(v)

    walk(bir)
    return bir


_orig_to_json_bytes = bass.Bass.to_json_bytes


def _patched_to_json_bytes(self):
    import json as _json
    raw = _orig_to_json_bytes(self)
    bir = _json.loads(raw)
    bir = _split_excess_waits(bir)
    return _json.dumps(bir).encode()


bass.Bass.to_json_bytes = _patched_to_json_bytes

N, M, D = 131072, 32, 32
NCORES = 8
NC_N = N // NCORES          # points per core
CHUNK = 512                 # points per n-chunk (one fp32 matmul free-dim)
NCHUNK = NC_N // CHUNK      # 32
NBLK = (M * D) // 128       # 8 blocks of 128 (m,k) pairs
XTILE = 4                   # xta split into 4 SBUF tiles for load/compute overlap
LAG = 3                     # software pipeline lag (w-matmul ahead of square)

F32 = mybir.dt.float32
F32R = mybir.dt.float32r


def _build_program():
    nc = bass.Bass("TRN2", debug=False)

    xta_d = nc.dram_tensor("xta", [D + 1, NC_N], F32R, kind="ExternalInput").ap()
    lw_d = nc.dram_tensor("lw", [D, NBLK, 128], F32R, kind="ExternalInput").ap()
    acw2_d = nc.dram_tensor("acw2", [D + 1, M], F32R, kind="ExternalInput").ap()
    sel_d = nc.dram_tensor("sel", [128, NBLK, M], F32R, kind="ExternalInput").ap()
    ones_d = nc.dram_tensor("onesw", [M, 1], F32R, kind="ExternalInput").ap()
    out_d = nc.dram_tensor("out", [1, NC_N], F32, kind="ExternalOutput").ap()

    with tile.TileContext(nc) as tc, ExitStack() as ctx:
        singles = ctx.enter_context(tc.tile_pool(name="singles", bufs=1))
        xpool = ctx.enter_context(tc.tile_pool(name="xpool", bufs=1))
        u2pool = ctx.enter_context(tc.tile_pool(name="u2", bufs=LAG + 2))
        epool = ctx.enter_context(tc.tile_pool(name="exp", bufs=2))
        wps = ctx.enter_context(tc.tile_pool(name="wps", bufs=LAG + 1, space="PSUM"))
        dps = ctx.enter_context(tc.tile_pool(name="dps", bufs=2, space="PSUM"))
        fps = ctx.enter_context(tc.tile_pool(name="fps", bufs=2, space="PSUM"))

        lw_sb = singles.tile([D, NBLK, 128], F32R)
        nc.sync.dma_start(out=lw_sb, in_=lw_d)
        acw2_sb = singles.tile([D + 1, M], F32R)
        nc.sync.dma_start(out=acw2_sb, in_=acw2_d)
        sel_sb = singles.tile([128, NBLK, M], F32R)
        nc.sync.dma_start(out=sel_sb, in_=sel_d)
        ones_sb = singles.tile([M, 1], F32R)
        nc.sync.dma_start(out=ones_sb, in_=ones_d)
        osb = singles.tile([1, NC_N], F32)

        xtiles = []
        xw = NC_N // XTILE
        for i in range(XTILE):
            xt = xpool.tile([D + 1, xw], F32R, name=f"xt{i}")
            nc.sync.dma_start(out=xt, in_=xta_d[:, i * xw:(i + 1) * xw])
            xtiles.append(xt)

        def xslice(c, rows):
            ti, off = divmod(c * CHUNK, xw)
            return xtiles[ti][0:rows, off:off + CHUNK]

        TOT = NCHUNK * NBLK
        wtiles = {}
        d_ps = None
        sq_ctr = 0
        for t in range(TOT + LAG):
            if t < TOT:
                c, b = divmod(t, NBLK)
                w = wps.tile([128, CHUNK], F32, name="w")
                nc.tensor.matmul(w, lw_sb[:, b, :], xslice(c, D),
                                 start=True, stop=True)
                wtiles[t] = w
            if t >= LAG:
                t2 = t - LAG
                c2, b2 = divmod(t2, NBLK)
                w = wtiles.pop(t2)
                u2 = u2pool.tile([128, CHUNK], F32R, name="u2")
                # PSUM evacuation + square, split ACT (~70%) / DVE (~30%).
                # DVE cannot read PSUM twice, so it copies once and multiplies
                # the SBUF copy against the PSUM operand.
                if sq_ctr % 10 < 3:
                    u_sb = u2pool.tile([128, CHUNK], F32R, name="ucp")
                    nc.vector.tensor_copy(u_sb, w)
                    nc.vector.tensor_mul(u2, w, u_sb)
                else:
                    nc.scalar.square(u2, w)
                sq_ctr += 1
                if b2 == 0:
                    d_ps = dps.tile([M, CHUNK], F32, name="d")
                nc.tensor.matmul(d_ps, sel_sb[:, b2, :], u2,
                                 start=(b2 == 0), stop=False,
                                 skip_group_check=True)
                if b2 == NBLK - 1:
                    nc.tensor.matmul(d_ps, acw2_sb, xslice(c2, D + 1),
                                     start=False, stop=True,
                                     skip_group_check=True)
                    e_sb = epool.tile([M, CHUNK], F32R, name="e")
                    nc.scalar.activation(e_sb, d_ps,
                                         mybir.ActivationFunctionType.Exp)
                    f_ps = fps.tile([1, CHUNK], F32, name="f")
                    nc.tensor.matmul(f_ps, ones_sb, e_sb,
                                     start=True, stop=True)
                    nc.scalar.copy(
                        osb[0:1, c2 * CHUNK:(c2 + 1) * CHUNK], f_ps)

        nc.sync.dma_start(out=out_d, in_=osb)
    return nc


_CACHE = {}


def _get_program():
    if "nc" not in _CACHE:
        _CACHE["nc"] = _build_program()
    return _CACHE["nc"]


def _host_prep(points, centers, covs_inv_sqrt, weights):
    L = covs_inv_sqrt.astype(np.float64)                    # [M, D, D]
    A = np.einsum("mdk,mek->mde", L, L)                     # [M, D, D]
    Ac = np.einsum("mde,me->md", A, centers.astype(np.float64))
    cAc = np.einsum("md,md->m", Ac, centers.astype(np.float64))
    wf = weights.astype(np.float64)
    sm = np.exp(wf - wf.max())
    sm /= sm.sum()
    coefs = sm * np.sqrt(np.linalg.det(A))
    lnc = np.log(coefs)

    lw = np.ascontiguousarray(
        L.transpose(1, 0, 2).reshape(D, NBLK, 128)).astype(np.float32)
    acw2 = np.zeros((D + 1, M), np.float32)
    acw2[:D, :] = (2.0 * Ac.T).astype(np.float32)
    acw2[D, :] = (-cAc + lnc).astype(np.float32)
    sel = np.zeros((128, NBLK, M), np.float32)
    p = np.arange(128)
    for b in range(NBLK):
        sel[p, b, 4 * b + p // 32] = -1.0
    onesw = np.ones((M, 1), np.float32)

    xta = np.empty((D + 1, N), np.float32)
    xta[:D, :] = points.T
    xta[D, :] = 1.0
    return xta, lw, acw2, sel, onesw


def kernel(points, centers, covs_inv_sqrt, weights):
    xta, lw, acw2, sel, onesw = _host_prep(points, centers, covs_inv_sqrt, weights)
    nc = _get_program()
    in_maps = []
    for i in range(NCORES):
        in_maps.append({
            "xta": np.ascontiguousarray(xta[:, i * NC_N:(i + 1) * NC_N]),
            "lw": lw, "acw2": acw2, "sel": sel, "onesw": onesw,
        })
    trace = os.environ.get("KBENCH_TRACE", "") == "1"
    if trace:
        try:
            from antenv.axon_hooks import get_axon_ntff_profile_hook  # noqa: F401
        except ImportError:
            trace = False
    res = run_bass_kernel_spmd(nc, in_maps, core_ids=list(range(NCORES)),
                               trace=trace)
    _CACHE["last_result"] = res
    out = np.concatenate([res.results[i]["out"].reshape(-1)
                          for i in range(NCORES)])
    return out.astype(np.float32)
